# revision 9
# baseline (speedup 1.0000x reference)
"""Block floating-point quantizer (DMX BFP: PRECISION=8, BLOCK_SIZE=128) on 8
Trainium2 NeuronCores.

Math (per 128-elem block along the last dim):
    amax  = max(|x_block|)
    e     = floor(log2(amax))
    scale = 2^(e-6)
    y     = clip(round_half_even(x/scale), -127, 127) * scale

Implemented without division/log/exp via exact fp32 tricks:
    p2  = 2^e, recovered exactly by masking the fp32 exponent field of amax
    C   = 1.5*2^23*scale = p2 * 196608.0          (exact product)
    b   = C*K3 with K3 ~= 127/(1.5*2^23): any b in (126.5, 127.5)*scale
          gives output identical to clamping q to [-127, 127]
    y   = (clamp(x, -b, b) + C) - C   -- fp32 RNE rounds to a multiple of
          scale; the subtract is exact

An all-zero block needs no guard: amax = 0 gives p2 = 0, C = 0, b = 0,
and the fused op returns (clamp(x,0,0)+0)-0 = 0, the correct output.

Every output value q*scale (|q| <= 127) has at most 7 significant mantissa
bits, so it is EXACTLY representable in bfloat16: the kernel stores the
output as bf16 (halving write-side HBM traffic) and the host upcasts back
to fp32 bit-exactly.

Engine split per [128, 8192] row-tile (the 16 SDMA engines at ~27 GB/s
each are the roofline; DVE runs just under it):
  - ACT (scalar) engine: xa = |x| as bf16 (Abs activation w/ output
    downconvert) -- feeds the reduce, off the critical DVE path.
  - DVE: per-block amax via a pairwise-max tree on bf16 (2x_1p perf
    mode; tensor_reduce would be 1x-only); the last tree level, the
    exponent-field mask and the *C_MULT are fused into one 3-stage
    custom-DVE op emitting the per-block C stream directly. Then ONE
    fused custom-DVE quant instruction (min, neg, max, add, sub in a
    single 8-stage pass) over the fp32 tile, emitting bf16 directly.
  - DMA: inputs ride the SP HWDGE ring; outputs ride gpsimd SWDGE.
The first and last row-tiles are processed in 2048-column chunks to
shorten pipeline fill (first abs available ~4x sooner) and drain (the
final quant's DVE pipe flush scales with its width).
"""

import sys

for _p in ("/opt/trn_rl_repo",):
    if _p not in sys.path:
        sys.path.insert(0, _p)

import numpy as np

import concourse.bacc as bacc
import concourse.tile as tile
from concourse import mybir
from concourse import dve_ops as _dve_ops
from concourse.bass_utils import run_bass_kernel_spmd
from concourse.dve_ops import DveOp, has_src1
from concourse.dve_spec import C0, C1, Spec, Src0, Src1, Zero, maxx, minn
from concourse.dve_spec import Bin, AluOp
from concourse.dve_spec import lower as _dve_lower
from concourse.dve_uop import DveOpSpec

N_CORES = 8
ROWS, COLS = 8192, 8192
SHARD_ROWS = ROWS // N_CORES  # 1024
P = 128                       # SBUF partitions
BLK = 128                     # shared-exponent block size

EXP_MASK = 0x7F800000         # fp32 exponent-field mask
C_MULT = 196608.0             # 1.5 * 2^17: p2 * C_MULT == 1.5*2^23*scale, exact
K3 = float(np.float32(127.0 / (1.5 * 2**23)))
#                             # C*K3 ~= 127*scale, well inside (126.5, 127.5)*scale

_F32 = mybir.dt.float32
_BF16 = mybir.dt.bfloat16
_I32 = mybir.dt.int32


def _register_dve_op(name, spec):
    """Register a custom DVE op in the module-level tables at runtime
    (same three structures dve_ops.py populates at import)."""
    for op in _dve_ops.OPS:
        if op.name == name:
            return op
    row = _dve_ops._CUSTOM_DVE_ROW_BASE + len(_dve_ops.OPS)
    assert row < 0x20, "custom-DVE row field overflow"
    _dve_ops._SUB_OPCODE_FOR_NAME[name] = row
    shas = {}
    for ver in ("v3", "v4"):
        uops = _dve_lower(spec, ver=ver)
        shas[ver] = DveOpSpec(
            name=name, opcode=row, uops=uops, rd1_en=has_src1(spec)
        ).sha(ver)
    op = DveOp(name, spec, subdim=False, uops_sha=shas)
    _dve_ops.OPS.append(op)
    _dve_ops.CUSTOM_DVE_SPECS[name] = spec
    return op


def _match(in0, in1):
    # CoreSim may hand in0 as the coalesced view while in1 keeps its
    # 3D broadcast shape; reconcile to in1's shape (same element order).
    if in1 is not None and in0.shape != in1.shape:
        in0 = in0.reshape(in1.shape)
    return in0, in1


def _quant_ref(in0, in1, c0, c1, c2):
    # in0 = x, in1 = C stream, c1 = K3. fp32 throughout:
    #   b  = C*K3   (any value in (126.5, 127.5)*scale is correct)
    #   xc = clamp(x, -b, b); y = (xc + C) - C  (RNE between the ops)
    in0, in1 = _match(in0, in1)
    f32 = np.float32
    b = (in1 * f32(c1)).astype(f32)
    xc = np.maximum(np.minimum(in0, b), (f32(0.0) - b).astype(f32))
    t = (xc + in1).astype(f32)
    return (t - in1).astype(f32)


_m1 = Src1 * C1
BFP_QUANT = _register_dve_op(
    "BFP_QUANT_ANT",
    Spec(
        body=(maxx(minn(Src0, _m1), Zero - _m1) + Src1) - Src1,
        reference=_quant_ref,
    ),
)


def _cmag_ref(in0, in1, c0, c1, c2):
    # Last tree level + exponent-field extraction + *C_MULT in one op:
    #   cmag = bits(max(a, b)) & 0x7F800000, reinterpreted fp32, * c1
    # c0 carries the mask as an fp32 bit pattern (+inf); ignored here.
    in0, in1 = _match(in0, in1)
    m = np.maximum(in0, in1).astype(np.float32)
    p2 = (m.view(np.int32) & np.int32(EXP_MASK)).view(np.float32)
    return (p2 * np.float32(c1)).astype(np.float32)


BFP_CMAG = _register_dve_op(
    "BFP_CMAG_ANT",
    Spec(
        body=Bin(
            AluOp.MULTIPLY,
            Bin(AluOp.BITWISE_AND, maxx(Src0, Src1), C0),
            C1,
        ),
        reference=_cmag_ref,
    ),
)


def build(
    shard_rows=SHARD_ROWS,
    cols=COLS,
    tile_cols=8192,
    edge_cols=2048,
    io_bufs=3,
    swq=2,
    yt_bufs=2,
    xa_bufs=2,
    tree_bufs=2,
    cmag_fuse=1,
    out_split=0,
):
    tile_cols = min(tile_cols, cols)
    nc = bacc.Bacc("TRN2", target_bir_lowering=False, num_swdge_queues=swq)
    x = nc.declare_dram_parameter("x", [shard_rows, cols], _F32, isOutput=False)
    y = nc.declare_dram_parameter("out", [shard_rows, cols], _BF16, isOutput=True)

    row_tiles = shard_rows // P

    def chunks_for(it):
        w = edge_cols if it in (0, row_tiles - 1) and edge_cols else tile_cols
        return [(co, w) for co in range(0, cols, w)]

    with tile.TileContext(nc) as tc:
        with (
            tc.tile_pool(name="io", bufs=io_bufs) as io_pool,
            tc.tile_pool(name="oy", bufs=yt_bufs) as oy_pool,
            tc.tile_pool(name="xa", bufs=xa_bufs) as xa_pool,
            tc.tile_pool(name="tree", bufs=tree_bufs) as tree_pool,
            tc.tile_pool(name="small", bufs=3) as small_pool,
            tc.tile_pool(name="const", bufs=1) as const_pool,
        ):
            # Exponent-field mask delivered as a per-partition fp32 scalar
            # (bit pattern 0x7F800000 == +inf) for the fused cmag op.
            mask_f = const_pool.tile([P, 1], _I32, tag="maskf")
            nc.vector.memset(mask_f[:], EXP_MASK)

            oi = 0
            for it in range(row_tiles):
                rs = slice(it * P, (it + 1) * P)
                xt = io_pool.tile([P, cols], _F32, tag="xt")
                xa = xa_pool.tile([P, cols], _BF16, tag="xa")
                yt = oy_pool.tile([P, cols], _BF16, tag="yt")
                for co, w in chunks_for(it):
                    cs = slice(co, co + w)
                    nblk_t = w // BLK
                    nc.sync.dma_start(out=xt[:, cs], in_=x[rs, cs])

                    # ACT engine: |x| downconverted to bf16.
                    nc.scalar.activation(
                        out=xa[:, cs],
                        in_=xt[:, cs],
                        func=mybir.ActivationFunctionType.Abs,
                    )

                    # DVE pairwise-max tree on bf16 (2x_1p perf mode).
                    cur = xa[:, cs].rearrange("p (b k) -> p b k", k=BLK)
                    s = BLK // 2
                    while s >= 2:
                        m = tree_pool.tile(
                            [P, nblk_t, s], _BF16, tag=f"m{s}x{nblk_t}"
                        )
                        nc.vector.tensor_tensor(
                            out=m[:],
                            in0=cur[:, :, 0:s],
                            in1=cur[:, :, s : 2 * s],
                            op=mybir.AluOpType.max,
                        )
                        cur = m[:]
                        s //= 2

                    cmag = small_pool.tile(
                        [P, nblk_t], _F32, tag=f"cmag{nblk_t}"
                    )
                    if cmag_fuse:
                        # max(a,b) -> &0x7F800000 -> *C_MULT in one op.
                        nc.vector._custom_dve(
                            BFP_CMAG,
                            out=cmag[:],
                            in0=cur[:, :, 0],
                            in1=cur[:, :, 1],
                            s0=mask_f[:].bitcast(_F32),
                            s1=C_MULT,
                        )
                    else:
                        amax = small_pool.tile(
                            [P, nblk_t], _F32, tag=f"amax{nblk_t}"
                        )
                        nc.vector.tensor_tensor(
                            out=amax[:].unsqueeze(2),
                            in0=cur[:, :, 0:1],
                            in1=cur[:, :, 1:2],
                            op=mybir.AluOpType.max,
                        )
                        p2 = small_pool.tile(
                            [P, nblk_t], _I32, tag=f"p2{nblk_t}"
                        )
                        nc.vector.tensor_tensor(
                            out=p2[:],
                            in0=amax[:].bitcast(_I32),
                            in1=mask_f[:].to_broadcast((P, nblk_t)),
                            op=mybir.AluOpType.bitwise_and,
                        )
                        nc.vector.tensor_scalar_mul(
                            cmag[:], p2[:].bitcast(_F32), C_MULT
                        )

                    x3 = xt[:, cs].rearrange("p (b k) -> p b k", k=BLK)
                    y3 = yt[:, cs].rearrange("p (b k) -> p b k", k=BLK)
                    c3 = cmag[:].unsqueeze(2).to_broadcast((P, nblk_t, BLK))
                    nc.vector._custom_dve(
                        BFP_QUANT, out=y3, in0=x3, in1=c3, s1=K3
                    )

                    if out_split == 2 and (oi % 2 == 1):
                        nc.scalar.dma_start(out=y[rs, cs], in_=yt[:, cs])
                    else:
                        nc.gpsimd.dma_start(out=y[rs, cs], in_=yt[:, cs])
                    oi += 1

    nc.compile()
    return nc


_nc_cache = {}


def _get_nc():
    if "nc" not in _nc_cache:
        _nc_cache["nc"] = build()
    return _nc_cache["nc"]


def kernel(x):
    x = np.ascontiguousarray(np.asarray(x, dtype=np.float32))
    assert x.shape == (ROWS, COLS)
    nc = _get_nc()
    in_maps = [
        {"x": x[i * SHARD_ROWS : (i + 1) * SHARD_ROWS]} for i in range(N_CORES)
    ]
    res = run_bass_kernel_spmd(nc, in_maps, core_ids=list(range(N_CORES)))
    # bf16 -> fp32 upcast is exact: every q*scale (|q| <= 127) has <= 7
    # significant mantissa bits.
    return np.concatenate(
        [np.asarray(r["out"]).astype(np.float32) for r in res.results], axis=0
    )
